# revision 24
# baseline (speedup 1.0000x reference)
"""Trainium2 Bass kernel for nn_LongRangeFeaturizer (Ewald sum featurizer).

Shards the 16 independent systems across 8 NeuronCores (2 systems/core).
All heavy math (charges matmul, k-space structure factors, trig, short-range
erf/cutoff coefficients, scatter, final combine) runs on-device.

Key structure (v4):
 - k-grid truncated to |n|^2 <= 15 (125 half-grid vectors + 1 background
   slot = one 128-wide k-tile); truncation error ~1.8e-3 relative, well
   under the fp16 noise floor (G ~ exp(-k^2/2)/k^2 decays brutally fast).
 - trig computed once in [K, 2N] layout (sin via table; cos = 1-2sin^2(pi v)
   on DVE); the [N, K] layout for stage 1 comes from PE transposes.
 - stage 1 computed k-major (S[k, d]); G multiply is then a per-partition
   scalar op and stage 2 consumes S directly - no extra transposes.
 - erf via tanh(a d + b d^3) (gelu identity, |err| < 4e-4): the whole kernel
   then uses one activation table set (sin/tanh/copy) = one table load.
 - Ewald self term folded into the short-range scatter matrix as diagonal
   edges with d ~ 0: sr(d->0) = -sqrt(2/pi)/sigma exactly.
 - background (k=0) term folded into the padded k slot with G = -pi s^2/V.
   The final combine is then a single multiply: out = pot * q.
 - short-range: first occurrence of each (j,i) edge goes through 512-wide
   local_scatter (8 calls, the only Pool work); duplicate occurrences
   (~530/system) are handled as 128-edge chunks: gather the edge's features
   (host-arranged columns), matmul to charges, scale rows by sr(d), then
   matmul against a host-built one-hot [slot -> i] matrix, accumulating
   straight into the potential PSUM.
"""

import sys

sys.path.insert(0, "/opt/trn_rl_repo")

import numpy as np

import concourse.bass as bass
import concourse.mybir as mybir
import concourse.tile as tile
from concourse import bacc, bass_utils

dt = mybir.dt
F32, F16, I16 = dt.float32, dt.float16, dt.int16
AF = mybir.ActivationFunctionType
AOP = mybir.AluOpType

PI = float(np.pi)
MAGIC = float(1.5 * 2**23)  # round-to-nearest-int magic constant for fp32

# Problem constants
S, N, D, E = 16, 512, 64, 16384
LCELL = 8.0
SMEAR = 1.0
EXCL = 5.0
LRWL = 1.0
PREF = 1.0
NMAX = 8
NCORES = 8
SYS_PER_CORE = S // NCORES

NSQ_CUT = 15          # keep |n|^2 <= 15; truncation err ~1.8e-3 rel
K2 = 128              # padded half-grid size (one k-tile)
DIAG_DIST = 0.01      # sr(0.01) ~= -sqrt(2/pi) = -selfc
PAD_DIST = float(EXCL)  # fcut(EXCL) = 0 -> padded slots contribute ~0
# erf(d/sqrt2) ~= tanh(TA*d + TB*d^3)  (gelu-tanh identity)
TA = 0.7978845608
TB = 0.0356774081

_CACHE = {}


def _half_kgrid():
    r = np.arange(-NMAX, NMAX + 1)
    n = np.stack(np.meshgrid(r, r, r, indexing="ij"), -1).reshape(-1, 3)
    n = n[np.any(n != 0, axis=1)]
    nsq = (n * n).sum(1)
    n = n[nsq <= NSQ_CUT]
    pos = (n[:, 0] > 0) | ((n[:, 0] == 0) & (n[:, 1] > 0)) | (
        (n[:, 0] == 0) & (n[:, 1] == 0) & (n[:, 2] > 0)
    )
    return n[pos].astype(np.int64)  # [K0, 3]


def _sr_arrange(nidx, ndist):
    """Per-system edge split: first occurrence of each (j,i) (incl. appended
    diagonal self edges) vs duplicates. Returns per-system (l0, l12) edge
    arrays and the widths R0 (l0 slots/row) and NCH (dup chunks/system)."""
    per_sys = []
    R0 = 0
    NCH = 1
    for s in range(S):
        j_t = np.concatenate([nidx[s, :, 1].astype(np.int64), np.arange(N)])
        i_t = np.concatenate([nidx[s, :, 0].astype(np.int64), np.arange(N)])
        d_t = np.concatenate([ndist[s].astype(np.float64),
                              np.full(N, DIAG_DIST)])
        cid = j_t * N + i_t
        order = np.argsort(cid, kind="stable")
        cs, js, is_, ds_ = cid[order], j_t[order], i_t[order], d_t[order]
        first = np.concatenate([[0], np.nonzero(np.diff(cs))[0] + 1])
        run_id = np.zeros(len(cs), np.int64)
        run_id[first] = 1
        run_id = np.cumsum(run_id) - 1
        occ = np.arange(len(cs)) - first[run_id]
        sel0 = occ == 0
        l0 = (js[sel0], is_[sel0], ds_[sel0])
        l12 = (js[~sel0], is_[~sel0], ds_[~sel0])
        per_sys.append((l0, l12))
        R0 = max(R0, int(np.bincount(l0[0], minlength=N).max()))
        NCH = max(NCH, -(-len(l12[0]) // 128))
    R0 += R0 % 2
    return per_sys, R0, NCH


def _build_nc(R0, NCH, reps=1):
    """Build the per-core SPMD program. NCH = dup chunks per system."""
    nc = bacc.Bacc("TRN2", target_bir_lowering=False, debug=False,
                   num_devices=NCORES)

    for val in (PI / 2,):
        t = nc.alloc_sbuf_tensor(f"constap-{val}", [128, 1], F32)
        nc.gpsimd.memset(t.ap(), val)
        nc.const_aps.aps[(F32, val)] = t.ap()
    nc.all_engine_barrier()

    def din(name, shape, d=F16):
        return nc.dram_tensor(name, shape, d, kind="ExternalInput").ap()

    SC = SYS_PER_CORE
    NCH2 = SC * NCH
    WSR = 8 * R0 + NCH2
    srd = din("srd", [128, WSR])              # f16 slot distances
    p6n6 = din("p6n6", [6, SC * N + K2])      # f16 [pT6 | nt6]
    sri = din("sri", [128, 8 * R0], I16)      # i16 l0 column indices
    nid = din("nid", [128, 256 + D])          # f16 [-I | I | WT]
    featT = din("featT", [D + 1, SC * N])     # f16 (features.T ; 1)
    gcol = din("gcol", [128, 2], F32)         # f32 G column (per-k scalar)
    featC = din("featC", [D + 1, NCH2 * 128])  # f16 dup-chunk features
    oneh = din("oneh", [128, NCH2 * N])       # f16 one-hot slot->i
    out = nc.dram_tensor("out", [SC * D, N], F32, kind="ExternalOutput").ap()

    NT = N // 128   # 4 atom tiles per system

    from contextlib import nullcontext
    with tile.TileContext(nc) as tc:
        with (
            tc.tile_pool(name="const", bufs=1) as cp,
            tc.tile_pool(name="work", bufs=2) as wp,
            tc.tile_pool(name="keep", bufs=1) as tp,
            tc.tile_pool(name="psU", bufs=2, space="PSUM") as pU,
            tc.tile_pool(name="psT", bufs=2, space="PSUM") as pT,
            tc.tile_pool(name="psH", bufs=1, space="PSUM") as pH,
            tc.For_i(0, reps, 1) if reps > 1 else nullcontext(),
        ):
            # ---- input DMAs (order = HWDGE serialization order) ----
            t_srd = cp.tile([128, WSR], F16, tag="srd")
            nc.sync.dma_start(out=t_srd[:], in_=srd[:])
            t_p6n6 = cp.tile([6, SC * N + K2], F16, tag="p6")
            nc.sync.dma_start(out=t_p6n6[:], in_=p6n6[:])
            t_sri = cp.tile([128, 8 * R0], I16, tag="sri")
            nc.sync.dma_start(out=t_sri[:], in_=sri[:])
            t_nid = cp.tile([128, 256 + D], F16, tag="nid")
            nc.sync.dma_start(out=t_nid[:], in_=nid[:])
            t_feat = cp.tile([D + 1, SC * N], F16, tag="feat")
            nc.sync.dma_start(out=t_feat[:], in_=featT[:])
            t_gcol = cp.tile([128, 2], F32, tag="gcol")
            nc.sync.dma_start(out=t_gcol[:], in_=gcol[:])
            t_fC = cp.tile([D + 1, NCH2 * 128], F16, tag="fC")
            nc.sync.dma_start(out=t_fC[:], in_=featC[:])
            t_oneh = cp.tile([128, NCH2 * N], F16, tag="oneh")
            nc.sync.dma_start(out=t_oneh[:], in_=oneh[:])

            t_pT6 = t_p6n6[:, 0:SC * N]
            t_nt6 = t_p6n6[:, SC * N:]
            t_negI = t_nid[:, 0:128]
            t_id16 = t_nid[:, 128:256]
            t_WT = t_nid[0:D + 1, 256:256 + D]

            # ---- act-table preload: Silu only lives in silu_and_others,
            #      which also holds Sin/Tanh/Copy -> one load total ----
            t_dum = wp.tile([128, 1], F16, tag="dum")
            nc.scalar.activation(t_dum[:], nc.const_aps.aps[(F32, PI / 2)],
                                 AF.Silu)

            # ---- short-range coefficients (fp16, single act table set),
            #      two column halves so the first scatters start early ----
            # sr(d) = (erf(d/sqrt2)/d) * (-0.5 - 0.5*sin(pi*d/5 + pi/2))
            # erf(d/sqrt2) ~= tanh(TA*d + TB*d^3)
            t_sr = wp.tile([128, WSR], F16, tag="srv")
            t_M = [None] * 8
            halves = [(slice(0, 4 * R0), range(0, 4)),
                      (slice(4 * R0, WSR), range(4, 8))]
            for hsl_sr, blks in halves:
                t_d = t_srd[:, hsl_sr]
                HW_ = hsl_sr.stop - hsl_sr.start
                t_fc = wp.tile([128, HW_], F16, tag="srfc", name="t_fc")
                nc.scalar.activation(t_fc[:], t_d, AF.Sin,
                                     scale=float(PI / EXCL), bias=PI / 2)
                t_sq = wp.tile([128, HW_], F16, tag="srsq", name="t_sq")
                nc.vector.tensor_tensor(out=t_sq[:], in0=t_d, in1=t_d,
                                        op=AOP.mult)
                t_g = wp.tile([128, HW_], F16, tag="srg", name="t_g")
                nc.vector.tensor_scalar(out=t_g[:], in0=t_sq[:],
                                        scalar1=TB, scalar2=TA,
                                        op0=AOP.mult, op1=AOP.add)
                t_arg = wp.tile([128, HW_], F16, tag="srarg", name="t_arg")
                nc.vector.tensor_tensor(out=t_arg[:], in0=t_g[:], in1=t_d,
                                        op=AOP.mult)
                t_erf = wp.tile([128, HW_], F16, tag="srerf", name="t_erf")
                nc.scalar.activation(t_erf[:], t_arg[:], AF.Tanh)
                t_rec = wp.tile([128, HW_], F16, tag="srrec", name="t_rec")
                with nc.allow_low_precision(reason="fp16 sr coeffs, 2e-2 tol"):
                    nc.vector.reciprocal(t_rec[:], t_d)
                t_fc2 = wp.tile([128, HW_], F16, tag="srfc2", name="t_fc2")
                nc.vector.tensor_scalar(out=t_fc2[:], in0=t_fc[:],
                                        scalar1=-0.5 * PREF,
                                        scalar2=-0.5 * PREF,
                                        op0=AOP.mult, op1=AOP.add)
                t_m1 = wp.tile([128, HW_], F16, tag="srm1", name="t_m1")
                nc.vector.tensor_tensor(out=t_m1[:], in0=t_erf[:],
                                        in1=t_rec[:], op=AOP.mult)
                nc.vector.tensor_tensor(out=t_sr[:, hsl_sr], in0=t_m1[:],
                                        in1=t_fc2[:], op=AOP.mult)
                for blk in blks:
                    m = tp.tile([128, N], F16, tag=f"m_{blk}", name=f"m_{blk}")
                    csl = slice(blk * R0, (blk + 1) * R0)
                    nc.gpsimd.local_scatter(out_ap=m[:], data_ap=t_sr[:, csl],
                                            idxs_ap=t_sri[:, csl],
                                            channels=128, num_elems=N,
                                            num_idxs=R0)
                    t_M[blk] = m
            # fp32 copy of the dup-chunk sr columns (activation scale AP)
            t_srf = wp.tile([128, NCH2], F32, tag="srf")
            nc.vector.tensor_copy(out=t_srf[:], in_=t_sr[:, 8 * R0:])

            # ---- trig in KN layout: c,s [K2, 2N] f16, chunked per system ----
            # s = sin(2*pi*v), c = 1 - 2*sin(pi*v)^2 with v = u - round(u)
            t_c = tp.tile([128, SC * N], F16, tag="ckn")
            t_s = tp.tile([128, SC * N], F16, tag="skn")
            for h in range(SC):
                hsl = slice(h * N, h * N + N)
                ps_u = pU.tile([128, N], F32, tag="uc")
                nc.tensor.matmul(out=ps_u[:], lhsT=t_nt6[:],
                                 rhs=t_pT6[:, hsl], start=True, stop=False)
                t_r = wp.tile([128, N], F16, tag="rnd")
                nc.vector.tensor_scalar(out=t_r[:], in0=ps_u[:],
                                        scalar1=MAGIC, scalar2=MAGIC,
                                        op0=AOP.add, op1=AOP.subtract)
                nc.tensor.matmul(out=ps_u[:], lhsT=t_negI[:], rhs=t_r[:],
                                 start=False, stop=True)
                nc.scalar.activation(t_s[:, hsl], ps_u[:], AF.Sin,
                                     scale=2 * PI)
                ts2 = wp.tile([128, N], F16, tag="s2", bufs=2)
                nc.scalar.activation(ts2[:], ps_u[:], AF.Sin, scale=PI)
                t_sq2 = wp.tile([128, N], F16, tag="sq2")
                nc.vector.tensor_tensor(out=t_sq2[:], in0=ts2[:],
                                        in1=ts2[:], op=AOP.mult)
                nc.vector.tensor_scalar(out=t_c[:, hsl], in0=t_sq2[:],
                                        scalar1=-2.0, scalar2=1.0,
                                        op0=AOP.mult, op1=AOP.add)

            # ---- charges ----
            ps_qT = pH.tile([128, N], F32, tag="qt")
            for sys in range(SC):
                csl = slice(sys * N, sys * N + N)
                nc.tensor.matmul(out=ps_qT[sys * D:(sys + 1) * D, :],
                                 lhsT=t_WT[:], rhs=t_feat[:, csl],
                                 start=True, stop=True)
            t_qT = tp.tile([128, N], F32, tag="qtf")
            nc.scalar.activation(t_qT[:], ps_qT[:], AF.Copy)
            t_q16 = [[None] * NT for _ in range(SC)]
            for sys in range(SC):
                for nt_i in range(NT):
                    fsl = slice(sys * N + nt_i * 128, sys * N + nt_i * 128 + 128)
                    ps_q = pU.tile([128, N], F32, tag="uc")
                    nc.tensor.matmul(out=ps_q[:, 0:D], lhsT=t_feat[:, fsl],
                                     rhs=t_WT[:], start=True, stop=True)
                    tq = tp.tile([128, D], F16, tag=f"q16_{sys}_{nt_i}")
                    nc.scalar.activation(tq[:], ps_q[:, 0:D], AF.Copy)
                    t_q16[sys][nt_i] = tq
            # dup-chunk charges, scaled by sr(d) in the copy
            t_Y = []
            for gc in range(NCH2):
                ps_qc = pU.tile([128, N], F32, tag="uc")
                fsl = slice(gc * 128, (gc + 1) * 128)
                nc.tensor.matmul(out=ps_qc[:, 0:D], lhsT=t_fC[:, fsl],
                                 rhs=t_WT[:], start=True, stop=True)
                ty = tp.tile([128, D], F16, tag=f"y{gc}", name=f"y{gc}")
                nc.scalar.activation(ty[:], ps_qc[:, 0:D], AF.Copy,
                                     scale=t_srf[:, gc:gc + 1])
                t_Y.append(ty)

            # ---- per system: NK transposes -> stage1 (k-major) -> G ----
            ps_S = pH.tile([128, SC * 128], F32, tag="s1")
            t_cs = [[None] * NT for _ in range(SC)]
            t_GS = []
            for sys in range(SC):
                scol = slice(sys * 128, sys * 128 + 128)
                # NK tiles [128(n), 256] = [c | s]
                for nt_i in range(NT):
                    nsl = slice(sys * N + nt_i * 128, sys * N + nt_i * 128 + 128)
                    ps_tr = pT.tile([128, 256], F16, tag="tr")
                    nc.tensor.transpose(out=ps_tr[:, 0:128], in_=t_c[:, nsl],
                                        identity=t_id16[:])
                    nc.tensor.transpose(out=ps_tr[:, 128:256], in_=t_s[:, nsl],
                                        identity=t_id16[:])
                    tt_ = tp.tile([128, 256], F16, tag=f"cs{sys}_{nt_i}")
                    nc.vector.tensor_copy(out=tt_[:], in_=ps_tr[:])
                    t_cs[sys][nt_i] = tt_
                # stage1 k-major: S[k, d] = sum_n c/s[n,k] q[n,d]
                # sequential accumulation groups: column-interleaved groups
                # in one psum region lose contributions (see v4 post-mortem)
                for half in range(2):
                    csl2 = slice(sys * 128 + half * D,
                                 sys * 128 + (half + 1) * D)
                    for nt_i in range(NT):
                        nc.tensor.matmul(out=ps_S[:, csl2],
                                         lhsT=t_cs[sys][nt_i][:,
                                              half * 128:(half + 1) * 128],
                                         rhs=t_q16[sys][nt_i][:],
                                         start=(nt_i == 0),
                                         stop=(nt_i == NT - 1))
                # G multiply: per-partition (= per-k) scalar
                tg = tp.tile([128, 128], F16, tag=f"gs{sys}", name=f"gs{sys}")
                nc.vector.tensor_scalar(out=tg[:], in0=ps_S[:, scol],
                                        scalar1=t_gcol[:, 0:1], scalar2=None,
                                        op0=AOP.mult)
                t_GS.append(tg)

            # ---- stage2 + dup chunks + M@q into one psum; per-sys close ----
            ps_pot = pH.tile([128, N], F32, tag="pot")
            for sys in range(SC):
                csl = slice(sys * N, sys * N + N)
                orow = slice(sys * D, (sys + 1) * D)
                nc.tensor.matmul(out=ps_pot[orow, :],
                                 lhsT=t_GS[sys][:, 0:D], rhs=t_c[:, csl],
                                 start=True, stop=False)
                nc.tensor.matmul(out=ps_pot[orow, :],
                                 lhsT=t_GS[sys][:, D:128], rhs=t_s[:, csl],
                                 start=False, stop=False)
                for ch in range(NCH):
                    gc = sys * NCH + ch
                    nc.tensor.matmul(out=ps_pot[orow, :], lhsT=t_Y[gc][:],
                                     rhs=t_oneh[:, gc * N:(gc + 1) * N],
                                     start=False, stop=False)
                for jt in range(NT):
                    nc.tensor.matmul(out=ps_pot[orow, :],
                                     lhsT=t_q16[sys][jt][:],
                                     rhs=t_M[sys * NT + jt][:],
                                     start=False, stop=(jt == NT - 1))
                # combine: out = pot * q, then output DMA per system
                t_out = wp.tile([D, N], F32, tag=f"outf{sys}",
                                name=f"outf{sys}")
                nc.vector.tensor_tensor(out=t_out[:], in0=ps_pot[orow, :],
                                        in1=t_qT[orow, :], op=AOP.mult)
                nc.sync.dma_start(out=out[orow, :], in_=t_out[:])

    nc.compile()
    return nc


def _host_inputs(features, positions, cells, neighbor_indices,
                 neighbor_distances, W, b):
    features = np.asarray(features, np.float32)
    positions = np.asarray(positions, np.float32)
    cells = np.asarray(cells, np.float32)
    nidx = np.asarray(neighbor_indices)
    ndist = np.asarray(neighbor_distances, np.float32).reshape(S, E)
    W = np.asarray(W, np.float32)
    b = np.asarray(b, np.float32)

    assert np.allclose(cells, LCELL * np.eye(3, dtype=np.float32)[None]), \
        "kernel specialized to cubic L=8 cells"

    nh = _half_kgrid()
    K0 = len(nh)
    assert K0 <= K2 - 1
    ksq = (2.0 * PI / LCELL) ** 2 * (nh * nh).sum(1).astype(np.float64)
    vol = LCELL ** 3
    bgov = PREF * float(PI * SMEAR**2 / vol)
    G = 2.0 * PREF * (4.0 * PI / ksq) * np.exp(-0.5 * SMEAR**2 * ksq) / vol
    Gpad = np.zeros(K2, np.float64)
    Gpad[:K0] = G
    Gpad[K0] = -bgov  # background term via the k=0 pad slot (c=1, s=0)
    gcol = np.zeros((128, 2), np.float32)
    gcol[:, 0] = Gpad.astype(np.float32)

    per_sys, R0, NCH = _sr_arrange(nidx, ndist)
    SC = SYS_PER_CORE
    NCH2 = SC * NCH
    WSR = 8 * R0 + NCH2

    nt3 = np.zeros((3, K2), np.float16)
    nt3[:, :K0] = nh.T.astype(np.float16)
    nt6 = np.concatenate([nt3, nt3], 0)    # [6, K2]
    WT_aug = np.concatenate([W.T, b[None, :]], 0).astype(np.float16)  # [65, 64]
    nid = np.zeros((128, 256 + D), np.float16)
    nid[:, 0:128] = -np.eye(128)
    nid[:, 128:256] = np.eye(128)
    nid[0:D + 1, 256:256 + D] = WT_aug

    in_maps = []
    for core in range(NCORES):
        s0 = core * SC
        fa = []
        p6 = []
        for s in range(s0, s0 + SC):
            f = features[s * N:(s + 1) * N].T                      # [64, 512]
            fa.append(np.concatenate([f, np.ones((1, N), np.float32)], 0))
            pf = (positions[s].T.astype(np.float64)) / LCELL       # [3, 512]
            ph = pf.astype(np.float16)
            pl = (pf - ph.astype(np.float64)).astype(np.float16)
            p6.append(np.concatenate([ph, pl], 0))                 # [6, 512]
        p6n6 = np.concatenate(p6 + [nt6], 1).astype(np.float16)

        srd_c = np.full((128, WSR), PAD_DIST, np.float16)
        sri_c = np.full((128, 8 * R0), -1, np.int16)
        featC = np.zeros((D + 1, NCH2 * 128), np.float16)
        oneh = np.zeros((128, NCH2 * N), np.float16)
        for sys_local in range(SC):
            s = s0 + sys_local
            (js, is_, ds_), (js2, is2, ds2) = per_sys[s]
            cnt = np.bincount(js, minlength=N)
            start = np.concatenate([[0], np.cumsum(cnt)[:-1]])
            slot = np.arange(len(js)) - start[js]
            blk = sys_local * 4 + js // 128
            row = js % 128
            col = blk * R0 + slot
            srd_c[row, col] = ds_.astype(np.float16)
            sri_c[row, col] = is_.astype(np.int16)
            for e in range(len(js2)):
                ch = e // 128
                sl = e % 128
                gc = sys_local * NCH + ch
                srd_c[sl, 8 * R0 + gc] = np.float16(ds2[e])
                featC[0:D, gc * 128 + sl] = \
                    features[s * N + js2[e]].astype(np.float16)
                featC[D, gc * 128 + sl] = 1.0
                oneh[sl, gc * N + is2[e]] = 1.0

        m = {
            "srd": srd_c,
            "p6n6": p6n6,
            "sri": sri_c,
            "nid": nid,
            "featT": np.concatenate(fa, 1).astype(np.float16),
            "gcol": gcol,
            "featC": featC,
            "oneh": oneh,
        }
        in_maps.append(m)
    return in_maps, R0, NCH


def kernel(features, positions, cells, neighbor_indices, neighbor_distances,
           W, b, _trace=False):
    in_maps, R0, NCH = _host_inputs(features, positions, cells,
                                    neighbor_indices, neighbor_distances, W, b)
    key = (R0, NCH)
    if key not in _CACHE:
        _CACHE[key] = _build_nc(R0, NCH)
    nc = _CACHE[key]
    res = bass_utils.run_bass_kernel_spmd(nc, in_maps,
                                          core_ids=list(range(NCORES)),
                                          trace=_trace)
    blocks = []
    for i in range(NCORES):
        o = res.results[i]["out"]  # [SC*D, N] transposed per system
        for sys in range(SYS_PER_CORE):
            blocks.append(o[sys * D:(sys + 1) * D, :].T)
    out = np.concatenate(blocks, 0)
    if _trace:
        kernel.last_result = res
    return np.ascontiguousarray(out, dtype=np.float32)


def measure_hw_ns(features, positions, cells, neighbor_indices,
                  neighbor_distances, W, b, reps=300):
    """Time the kernel on hardware via an on-device repeat loop (amortizes
    the multi-ms axon RPC dispatch overhead). Returns per-iteration ns."""
    import time
    import jax
    from jax.sharding import Mesh, PartitionSpec, NamedSharding
    from jax.experimental.shard_map import shard_map
    from concourse import bass2jax
    from concourse.bass2jax import _bass_exec_p, partition_id_tensor

    bass2jax.install_neuronx_cc_hook()
    in_maps, R0, NCH = _host_inputs(features, positions, cells,
                                    neighbor_indices, neighbor_distances, W, b)

    def build_fn(nc, mesh, sh):
        partition_name = (nc.partition_id_tensor.name
                          if nc.partition_id_tensor else None)
        in_names, out_names, out_avals, zero_outs = [], [], [], []
        for alloc in nc.m.functions[0].allocations:
            if not isinstance(alloc, mybir.MemoryLocationSet):
                continue
            name = alloc.memorylocations[0].name
            if alloc.kind == "ExternalInput":
                if name != partition_name:
                    in_names.append(name)
            elif alloc.kind == "ExternalOutput":
                shape = tuple(alloc.tensor_shape)
                dtype = mybir.dt.np(alloc.dtype)
                out_names.append(name)
                out_avals.append(jax.core.ShapedArray(shape, dtype))
                zero_outs.append(np.zeros(shape, dtype))
        n_params = len(in_names)
        all_names = in_names + out_names
        if partition_name is not None:
            all_names = all_names + [partition_name]

        def _body(*args):
            operands = list(args)
            if partition_name is not None:
                operands.append(partition_id_tensor())
            return tuple(_bass_exec_p.bind(
                *operands, out_avals=tuple(out_avals), in_names=tuple(all_names),
                out_names=tuple(out_names), lowering_input_output_aliases=(),
                sim_require_finite=True, sim_require_nnan=True, nc=nc))

        specs_in = (PartitionSpec("core"),) * (n_params + len(out_names))
        specs_out = (PartitionSpec("core"),) * len(out_names)
        fn = jax.jit(shard_map(_body, mesh=mesh, in_specs=specs_in,
                               out_specs=specs_out, check_rep=False),
                     keep_unused=True)
        cat = [np.concatenate([np.asarray(in_maps[c][in_names[i]])
                               for c in range(NCORES)], 0)
               for i in range(n_params)]
        cat += [np.zeros((NCORES * z.shape[0], *z.shape[1:]), z.dtype)
                for z in zero_outs]
        dev = [jax.device_put(a, sh) for a in cat]
        return fn, dev

    devices = jax.devices()[:NCORES]
    mesh = Mesh(np.asarray(devices), ("core",))
    sh = NamedSharding(mesh, PartitionSpec("core"))

    def time_min(fn, dev, n=8):
        o = fn(*dev); jax.block_until_ready(o)
        best = float("inf")
        for _ in range(n):
            t0 = time.perf_counter()
            o = fn(*dev); jax.block_until_ready(o)
            best = min(best, (time.perf_counter() - t0) * 1e9)
        return best

    key1 = (R0, NCH)
    if key1 not in _CACHE:
        _CACHE[key1] = _build_nc(R0, NCH)
    fn1, dev1 = build_fn(_CACHE[key1], mesh, sh)
    t1 = time_min(fn1, dev1)
    keyr = (R0, NCH, reps)
    if keyr not in _CACHE:
        _CACHE[keyr] = _build_nc(R0, NCH, reps=reps)
    fnr, devr = build_fn(_CACHE[keyr], mesh, sh)
    tr = time_min(fnr, devr)
    return (tr - t1) / (reps - 1)


# revision 25
# speedup vs baseline: 1.1091x; 1.1091x over previous
"""Trainium2 Bass kernel for nn_LongRangeFeaturizer (Ewald sum featurizer).

Shards the 16 independent systems across 8 NeuronCores (2 systems/core).
All heavy math (charges matmul, k-space structure factors, trig, short-range
erf/cutoff coefficients, scatter, final combine) runs on-device.

Key structure (v4):
 - k-grid truncated to |n|^2 <= 15 (125 half-grid vectors + 1 background
   slot = one 128-wide k-tile); truncation error ~1.8e-3 relative, well
   under the fp16 noise floor (G ~ exp(-k^2/2)/k^2 decays brutally fast).
 - trig computed once in [K, 2N] layout (sin via table; cos = 1-2sin^2(pi v)
   on DVE); the [N, K] layout for stage 1 comes from PE transposes.
 - stage 1 computed k-major (S[k, d]); G multiply is then a per-partition
   scalar op and stage 2 consumes S directly - no extra transposes.
 - erf via tanh(a d + b d^3) (gelu identity, |err| < 4e-4): the whole kernel
   then uses one activation table set (sin/tanh/copy) = one table load.
 - Ewald self term folded into the short-range scatter matrix as diagonal
   edges with d ~ 0: sr(d->0) = -sqrt(2/pi)/sigma exactly.
 - background (k=0) term folded into the padded k slot with G = -pi s^2/V.
   The final combine is then a single multiply: out = pot * q.
 - short-range: first occurrence of each (j,i) edge goes through 512-wide
   local_scatter (8 calls, the only Pool work); duplicate occurrences
   (~530/system) are handled as 128-edge chunks: gather the edge's features
   (host-arranged columns), matmul to charges, scale rows by sr(d), then
   matmul against a host-built one-hot [slot -> i] matrix, accumulating
   straight into the potential PSUM.
"""

import sys

sys.path.insert(0, "/opt/trn_rl_repo")

import numpy as np

import concourse.bass as bass
import concourse.mybir as mybir
import concourse.tile as tile
from concourse import bacc, bass_utils

dt = mybir.dt
F32, F16, I16 = dt.float32, dt.float16, dt.int16
AF = mybir.ActivationFunctionType
AOP = mybir.AluOpType

PI = float(np.pi)
MAGIC = float(1.5 * 2**23)  # round-to-nearest-int magic constant for fp32

# Problem constants
S, N, D, E = 16, 512, 64, 16384
LCELL = 8.0
SMEAR = 1.0
EXCL = 5.0
LRWL = 1.0
PREF = 1.0
NMAX = 8
NCORES = 8
SYS_PER_CORE = S // NCORES

NSQ_CUT = 15          # keep |n|^2 <= 15; truncation err ~1.8e-3 rel
K2 = 128              # padded half-grid size (one k-tile)
DIAG_DIST = 0.01      # sr(0.01) ~= -sqrt(2/pi) = -selfc
PAD_DIST = float(EXCL)  # fcut(EXCL) = 0 -> padded slots contribute ~0
# erf(d/sqrt2) ~= tanh(TA*d + TB*d^3)  (gelu-tanh identity)
TA = 0.7978845608
TB = 0.0356774081

_CACHE = {}


def _half_kgrid():
    r = np.arange(-NMAX, NMAX + 1)
    n = np.stack(np.meshgrid(r, r, r, indexing="ij"), -1).reshape(-1, 3)
    n = n[np.any(n != 0, axis=1)]
    nsq = (n * n).sum(1)
    n = n[nsq <= NSQ_CUT]
    pos = (n[:, 0] > 0) | ((n[:, 0] == 0) & (n[:, 1] > 0)) | (
        (n[:, 0] == 0) & (n[:, 1] == 0) & (n[:, 2] > 0)
    )
    return n[pos].astype(np.int64)  # [K0, 3]


def _sr_arrange(nidx, ndist):
    """Per-system edge split: first occurrence of each (j,i) (incl. appended
    diagonal self edges) vs duplicates. Returns per-system (l0, l12) edge
    arrays and the widths R0 (l0 slots/row) and NCH (dup chunks/system)."""
    per_sys = []
    R0 = 0
    NCH = 1
    for s in range(S):
        j_t = np.concatenate([nidx[s, :, 1].astype(np.int64), np.arange(N)])
        i_t = np.concatenate([nidx[s, :, 0].astype(np.int64), np.arange(N)])
        d_t = np.concatenate([ndist[s].astype(np.float64),
                              np.full(N, DIAG_DIST)])
        cid = j_t * N + i_t
        order = np.argsort(cid, kind="stable")
        cs, js, is_, ds_ = cid[order], j_t[order], i_t[order], d_t[order]
        first = np.concatenate([[0], np.nonzero(np.diff(cs))[0] + 1])
        run_id = np.zeros(len(cs), np.int64)
        run_id[first] = 1
        run_id = np.cumsum(run_id) - 1
        occ = np.arange(len(cs)) - first[run_id]
        sel0 = occ == 0
        l0 = (js[sel0], is_[sel0], ds_[sel0])
        l12 = (js[~sel0], is_[~sel0], ds_[~sel0])
        per_sys.append((l0, l12))
        R0 = max(R0, int(np.bincount(l0[0], minlength=N).max()))
        NCH = max(NCH, -(-len(l12[0]) // 128))
    R0 += R0 % 2
    return per_sys, R0, NCH


def _build_nc(R0, NCH, reps=1):
    """Build the per-core SPMD program. NCH = dup chunks per system."""
    nc = bacc.Bacc("TRN2", target_bir_lowering=False, debug=False,
                   num_devices=NCORES)

    for val in (PI / 2,):
        t = nc.alloc_sbuf_tensor(f"constap-{val}", [128, 1], F32)
        nc.gpsimd.memset(t.ap(), val)
        nc.const_aps.aps[(F32, val)] = t.ap()
    nc.all_engine_barrier()

    def din(name, shape, d=F16):
        return nc.dram_tensor(name, shape, d, kind="ExternalInput").ap()

    SC = SYS_PER_CORE
    NCH2 = SC * NCH
    WSR = 8 * R0 + NCH2
    srd = din("srd", [128, WSR])              # f16 slot distances
    p6n6 = din("p6n6", [6, SC * N + K2])      # f16 [pT6 | nt6]
    sri = din("sri", [128, 8 * R0], I16)      # i16 l0 column indices
    nid = din("nid", [128, 256 + D])          # f16 [-I | I | WT]
    featT = din("featT", [D + 1, SC * N])     # f16 (features.T ; 1)
    gcol = din("gcol", [128, 2], F32)         # f32 G column (per-k scalar)
    featC = din("featC", [D + 1, NCH2 * 128])  # f16 dup-chunk features
    oneh = din("oneh", [128, NCH2 * N])       # f16 one-hot slot->i
    out = nc.dram_tensor("out", [SC * D, N], F32, kind="ExternalOutput").ap()

    NT = N // 128   # 4 atom tiles per system

    from contextlib import nullcontext
    with tile.TileContext(nc) as tc:
        with (
            tc.tile_pool(name="const", bufs=1) as cp,
            tc.tile_pool(name="work", bufs=2) as wp,
            tc.tile_pool(name="keep", bufs=1) as tp,
            tc.tile_pool(name="psU", bufs=2, space="PSUM") as pU,
            tc.tile_pool(name="psT", bufs=2, space="PSUM") as pT,
            tc.tile_pool(name="psH", bufs=1, space="PSUM") as pH,
            tc.For_i(0, reps, 1) if reps > 1 else nullcontext(),
        ):
            # ---- input DMAs (order = HWDGE serialization order) ----
            t_srd = cp.tile([128, WSR], F16, tag="srd")
            nc.sync.dma_start(out=t_srd[:], in_=srd[:])
            t_p6n6 = cp.tile([6, SC * N + K2], F16, tag="p6")
            nc.sync.dma_start(out=t_p6n6[:], in_=p6n6[:])
            t_sri = cp.tile([128, 8 * R0], I16, tag="sri")
            nc.sync.dma_start(out=t_sri[:], in_=sri[:])
            t_nid = cp.tile([128, 256 + D], F16, tag="nid")
            nc.sync.dma_start(out=t_nid[:], in_=nid[:])
            t_feat = cp.tile([D + 1, SC * N], F16, tag="feat")
            nc.sync.dma_start(out=t_feat[:], in_=featT[:])
            t_gcol = cp.tile([128, 2], F32, tag="gcol")
            nc.sync.dma_start(out=t_gcol[:], in_=gcol[:])
            t_fC = cp.tile([D + 1, NCH2 * 128], F16, tag="fC")
            nc.sync.dma_start(out=t_fC[:], in_=featC[:])
            t_oneh = cp.tile([128, NCH2 * N], F16, tag="oneh")
            nc.sync.dma_start(out=t_oneh[:], in_=oneh[:])

            t_pT6 = t_p6n6[:, 0:SC * N]
            t_nt6 = t_p6n6[:, SC * N:]
            t_negI = t_nid[:, 0:128]
            t_id16 = t_nid[:, 128:256]
            t_WT = t_nid[0:D + 1, 256:256 + D]

            # ---- act-table preload: Silu only lives in silu_and_others,
            #      which also holds Sin/Tanh/Copy -> one load total ----
            t_dum = wp.tile([128, 1], F16, tag="dum")
            nc.scalar.activation(t_dum[:], nc.const_aps.aps[(F32, PI / 2)],
                                 AF.Silu)

            # ---- short-range coefficients (fp16, single act table set) ----
            # sr(d) = (erf(d/sqrt2)/d) * (-0.5 - 0.5*sin(pi*d/5 + pi/2))
            # erf(d/sqrt2) ~= tanh(TA*d + TB*d^3)
            t_fc = wp.tile([128, WSR], F16, tag="srfc")
            nc.scalar.activation(t_fc[:], t_srd[:], AF.Sin,
                                 scale=float(PI / EXCL), bias=PI / 2)
            t_sq = wp.tile([128, WSR], F16, tag="srsq")
            nc.vector.tensor_tensor(out=t_sq[:], in0=t_srd[:], in1=t_srd[:],
                                    op=AOP.mult)
            t_g = wp.tile([128, WSR], F16, tag="srg")
            nc.vector.tensor_scalar(out=t_g[:], in0=t_sq[:],
                                    scalar1=TB, scalar2=TA,
                                    op0=AOP.mult, op1=AOP.add)
            t_arg = wp.tile([128, WSR], F16, tag="srarg")
            nc.vector.tensor_tensor(out=t_arg[:], in0=t_g[:], in1=t_srd[:],
                                    op=AOP.mult)
            t_erf = wp.tile([128, WSR], F16, tag="srerf")
            nc.scalar.activation(t_erf[:], t_arg[:], AF.Tanh)
            t_rec = wp.tile([128, WSR], F16, tag="srrec")
            with nc.allow_low_precision(reason="fp16 sr coefficients, 2e-2 tol"):
                nc.vector.reciprocal(t_rec[:], t_srd[:])
            t_fc2 = wp.tile([128, WSR], F16, tag="srfc2")
            nc.vector.tensor_scalar(out=t_fc2[:], in0=t_fc[:],
                                    scalar1=-0.5 * PREF, scalar2=-0.5 * PREF,
                                    op0=AOP.mult, op1=AOP.add)
            t_m1 = wp.tile([128, WSR], F16, tag="srm1")
            nc.vector.tensor_tensor(out=t_m1[:], in0=t_erf[:], in1=t_rec[:],
                                    op=AOP.mult)
            t_sr = wp.tile([128, WSR], F16, tag="srv")
            nc.vector.tensor_tensor(out=t_sr[:], in0=t_m1[:], in1=t_fc2[:],
                                    op=AOP.mult)
            # fp32 copy of the dup-chunk sr columns (activation scale AP)
            t_srf = wp.tile([128, NCH2], F32, tag="srf")
            nc.vector.tensor_copy(out=t_srf[:], in_=t_sr[:, 8 * R0:])

            # ---- 8 l0 scatters (sys-interleaved): M[blk] [128, 512] ----
            t_M = [None] * 8
            for jt in range(NT):
                for sys in range(SC):
                    blk = sys * 4 + jt
                    m = tp.tile([128, N], F16, tag=f"m_{blk}", name=f"m_{blk}")
                    csl = slice(blk * R0, (blk + 1) * R0)
                    nc.gpsimd.local_scatter(out_ap=m[:], data_ap=t_sr[:, csl],
                                            idxs_ap=t_sri[:, csl],
                                            channels=128, num_elems=N,
                                            num_idxs=R0)
                    t_M[blk] = m

            # ---- trig in KN layout: c,s [K2, 2N] f16, chunked per system ----
            # s = sin(2*pi*v), c = 1 - 2*sin(pi*v)^2 with v = u - round(u)
            t_c = tp.tile([128, SC * N], F16, tag="ckn")
            t_s = tp.tile([128, SC * N], F16, tag="skn")
            for h in range(SC):
                hsl = slice(h * N, h * N + N)
                ps_u = pU.tile([128, N], F32, tag="uc")
                nc.tensor.matmul(out=ps_u[:], lhsT=t_nt6[:],
                                 rhs=t_pT6[:, hsl], start=True, stop=False)
                t_r = wp.tile([128, N], F16, tag="rnd")
                nc.vector.tensor_scalar(out=t_r[:], in0=ps_u[:],
                                        scalar1=MAGIC, scalar2=MAGIC,
                                        op0=AOP.add, op1=AOP.subtract)
                nc.tensor.matmul(out=ps_u[:], lhsT=t_negI[:], rhs=t_r[:],
                                 start=False, stop=True)
                nc.scalar.activation(t_s[:, hsl], ps_u[:], AF.Sin,
                                     scale=2 * PI)
                ts2 = wp.tile([128, N], F16, tag="s2", bufs=2)
                nc.scalar.activation(ts2[:], ps_u[:], AF.Sin, scale=PI)
                t_sq2 = wp.tile([128, N], F16, tag="sq2")
                nc.vector.tensor_tensor(out=t_sq2[:], in0=ts2[:],
                                        in1=ts2[:], op=AOP.mult)
                nc.vector.tensor_scalar(out=t_c[:, hsl], in0=t_sq2[:],
                                        scalar1=-2.0, scalar2=1.0,
                                        op0=AOP.mult, op1=AOP.add)

            # ---- charges ----
            ps_qT = pH.tile([128, N], F32, tag="qt")
            for sys in range(SC):
                csl = slice(sys * N, sys * N + N)
                nc.tensor.matmul(out=ps_qT[sys * D:(sys + 1) * D, :],
                                 lhsT=t_WT[:], rhs=t_feat[:, csl],
                                 start=True, stop=True)
            t_qT = tp.tile([128, N], F32, tag="qtf")
            nc.scalar.activation(t_qT[:], ps_qT[:], AF.Copy)
            t_q16 = [[None] * NT for _ in range(SC)]
            for sys in range(SC):
                for nt_i in range(NT):
                    fsl = slice(sys * N + nt_i * 128, sys * N + nt_i * 128 + 128)
                    ps_q = pU.tile([128, N], F32, tag="uc")
                    nc.tensor.matmul(out=ps_q[:, 0:D], lhsT=t_feat[:, fsl],
                                     rhs=t_WT[:], start=True, stop=True)
                    tq = tp.tile([128, D], F16, tag=f"q16_{sys}_{nt_i}")
                    nc.scalar.activation(tq[:], ps_q[:, 0:D], AF.Copy)
                    t_q16[sys][nt_i] = tq
            # dup-chunk charges, scaled by sr(d) in the copy
            t_Y = []
            for gc in range(NCH2):
                ps_qc = pU.tile([128, N], F32, tag="uc")
                fsl = slice(gc * 128, (gc + 1) * 128)
                nc.tensor.matmul(out=ps_qc[:, 0:D], lhsT=t_fC[:, fsl],
                                 rhs=t_WT[:], start=True, stop=True)
                ty = tp.tile([128, D], F16, tag=f"y{gc}", name=f"y{gc}")
                nc.scalar.activation(ty[:], ps_qc[:, 0:D], AF.Copy,
                                     scale=t_srf[:, gc:gc + 1])
                t_Y.append(ty)

            # ---- per system: NK transposes -> stage1 (k-major) -> G ----
            ps_S = pH.tile([128, SC * 128], F32, tag="s1")
            t_cs = [[None] * NT for _ in range(SC)]
            t_GS = []
            for sys in range(SC):
                scol = slice(sys * 128, sys * 128 + 128)
                # NK tiles [128(n), 256] = [c | s]
                for nt_i in range(NT):
                    nsl = slice(sys * N + nt_i * 128, sys * N + nt_i * 128 + 128)
                    ps_tr = pT.tile([128, 256], F16, tag="tr")
                    nc.tensor.transpose(out=ps_tr[:, 0:128], in_=t_c[:, nsl],
                                        identity=t_id16[:])
                    nc.tensor.transpose(out=ps_tr[:, 128:256], in_=t_s[:, nsl],
                                        identity=t_id16[:])
                    tt_ = tp.tile([128, 256], F16, tag=f"cs{sys}_{nt_i}")
                    nc.vector.tensor_copy(out=tt_[:], in_=ps_tr[:])
                    t_cs[sys][nt_i] = tt_
                # stage1 k-major: S[k, d] = sum_n c/s[n,k] q[n,d]
                # sequential accumulation groups: column-interleaved groups
                # in one psum region lose contributions (see v4 post-mortem)
                for half in range(2):
                    csl2 = slice(sys * 128 + half * D,
                                 sys * 128 + (half + 1) * D)
                    for nt_i in range(NT):
                        nc.tensor.matmul(out=ps_S[:, csl2],
                                         lhsT=t_cs[sys][nt_i][:,
                                              half * 128:(half + 1) * 128],
                                         rhs=t_q16[sys][nt_i][:],
                                         start=(nt_i == 0),
                                         stop=(nt_i == NT - 1))
                # G multiply: per-partition (= per-k) scalar
                tg = tp.tile([128, 128], F16, tag=f"gs{sys}", name=f"gs{sys}")
                nc.vector.tensor_scalar(out=tg[:], in0=ps_S[:, scol],
                                        scalar1=t_gcol[:, 0:1], scalar2=None,
                                        op0=AOP.mult)
                t_GS.append(tg)

            # ---- stage2 + dup chunks + M@q into one psum; per-sys close ----
            ps_pot = pH.tile([128, N], F32, tag="pot")
            for sys in range(SC):
                csl = slice(sys * N, sys * N + N)
                orow = slice(sys * D, (sys + 1) * D)
                nc.tensor.matmul(out=ps_pot[orow, :],
                                 lhsT=t_GS[sys][:, 0:D], rhs=t_c[:, csl],
                                 start=True, stop=False)
                nc.tensor.matmul(out=ps_pot[orow, :],
                                 lhsT=t_GS[sys][:, D:128], rhs=t_s[:, csl],
                                 start=False, stop=False)
                for ch in range(NCH):
                    gc = sys * NCH + ch
                    nc.tensor.matmul(out=ps_pot[orow, :], lhsT=t_Y[gc][:],
                                     rhs=t_oneh[:, gc * N:(gc + 1) * N],
                                     start=False, stop=False)
                for jt in range(NT):
                    nc.tensor.matmul(out=ps_pot[orow, :],
                                     lhsT=t_q16[sys][jt][:],
                                     rhs=t_M[sys * NT + jt][:],
                                     start=False, stop=(jt == NT - 1))
                # combine: out = pot * q, then output DMA per system
                t_out = wp.tile([D, N], F32, tag=f"outf{sys}",
                                name=f"outf{sys}")
                nc.vector.tensor_tensor(out=t_out[:], in0=ps_pot[orow, :],
                                        in1=t_qT[orow, :], op=AOP.mult)
                nc.sync.dma_start(out=out[orow, :], in_=t_out[:])

    nc.compile()
    return nc


def _host_inputs(features, positions, cells, neighbor_indices,
                 neighbor_distances, W, b):
    features = np.asarray(features, np.float32)
    positions = np.asarray(positions, np.float32)
    cells = np.asarray(cells, np.float32)
    nidx = np.asarray(neighbor_indices)
    ndist = np.asarray(neighbor_distances, np.float32).reshape(S, E)
    W = np.asarray(W, np.float32)
    b = np.asarray(b, np.float32)

    assert np.allclose(cells, LCELL * np.eye(3, dtype=np.float32)[None]), \
        "kernel specialized to cubic L=8 cells"

    nh = _half_kgrid()
    K0 = len(nh)
    assert K0 <= K2 - 1
    ksq = (2.0 * PI / LCELL) ** 2 * (nh * nh).sum(1).astype(np.float64)
    vol = LCELL ** 3
    bgov = PREF * float(PI * SMEAR**2 / vol)
    G = 2.0 * PREF * (4.0 * PI / ksq) * np.exp(-0.5 * SMEAR**2 * ksq) / vol
    Gpad = np.zeros(K2, np.float64)
    Gpad[:K0] = G
    Gpad[K0] = -bgov  # background term via the k=0 pad slot (c=1, s=0)
    gcol = np.zeros((128, 2), np.float32)
    gcol[:, 0] = Gpad.astype(np.float32)

    per_sys, R0, NCH = _sr_arrange(nidx, ndist)
    SC = SYS_PER_CORE
    NCH2 = SC * NCH
    WSR = 8 * R0 + NCH2

    nt3 = np.zeros((3, K2), np.float16)
    nt3[:, :K0] = nh.T.astype(np.float16)
    nt6 = np.concatenate([nt3, nt3], 0)    # [6, K2]
    WT_aug = np.concatenate([W.T, b[None, :]], 0).astype(np.float16)  # [65, 64]
    nid = np.zeros((128, 256 + D), np.float16)
    nid[:, 0:128] = -np.eye(128)
    nid[:, 128:256] = np.eye(128)
    nid[0:D + 1, 256:256 + D] = WT_aug

    in_maps = []
    for core in range(NCORES):
        s0 = core * SC
        fa = []
        p6 = []
        for s in range(s0, s0 + SC):
            f = features[s * N:(s + 1) * N].T                      # [64, 512]
            fa.append(np.concatenate([f, np.ones((1, N), np.float32)], 0))
            pf = (positions[s].T.astype(np.float64)) / LCELL       # [3, 512]
            ph = pf.astype(np.float16)
            pl = (pf - ph.astype(np.float64)).astype(np.float16)
            p6.append(np.concatenate([ph, pl], 0))                 # [6, 512]
        p6n6 = np.concatenate(p6 + [nt6], 1).astype(np.float16)

        srd_c = np.full((128, WSR), PAD_DIST, np.float16)
        sri_c = np.full((128, 8 * R0), -1, np.int16)
        featC = np.zeros((D + 1, NCH2 * 128), np.float16)
        oneh = np.zeros((128, NCH2 * N), np.float16)
        for sys_local in range(SC):
            s = s0 + sys_local
            (js, is_, ds_), (js2, is2, ds2) = per_sys[s]
            cnt = np.bincount(js, minlength=N)
            start = np.concatenate([[0], np.cumsum(cnt)[:-1]])
            slot = np.arange(len(js)) - start[js]
            blk = sys_local * 4 + js // 128
            row = js % 128
            col = blk * R0 + slot
            srd_c[row, col] = ds_.astype(np.float16)
            sri_c[row, col] = is_.astype(np.int16)
            for e in range(len(js2)):
                ch = e // 128
                sl = e % 128
                gc = sys_local * NCH + ch
                srd_c[sl, 8 * R0 + gc] = np.float16(ds2[e])
                featC[0:D, gc * 128 + sl] = \
                    features[s * N + js2[e]].astype(np.float16)
                featC[D, gc * 128 + sl] = 1.0
                oneh[sl, gc * N + is2[e]] = 1.0

        m = {
            "srd": srd_c,
            "p6n6": p6n6,
            "sri": sri_c,
            "nid": nid,
            "featT": np.concatenate(fa, 1).astype(np.float16),
            "gcol": gcol,
            "featC": featC,
            "oneh": oneh,
        }
        in_maps.append(m)
    return in_maps, R0, NCH


def kernel(features, positions, cells, neighbor_indices, neighbor_distances,
           W, b, _trace=False):
    in_maps, R0, NCH = _host_inputs(features, positions, cells,
                                    neighbor_indices, neighbor_distances, W, b)
    key = (R0, NCH)
    if key not in _CACHE:
        _CACHE[key] = _build_nc(R0, NCH)
    nc = _CACHE[key]
    res = bass_utils.run_bass_kernel_spmd(nc, in_maps,
                                          core_ids=list(range(NCORES)),
                                          trace=_trace)
    blocks = []
    for i in range(NCORES):
        o = res.results[i]["out"]  # [SC*D, N] transposed per system
        for sys in range(SYS_PER_CORE):
            blocks.append(o[sys * D:(sys + 1) * D, :].T)
    out = np.concatenate(blocks, 0)
    if _trace:
        kernel.last_result = res
    return np.ascontiguousarray(out, dtype=np.float32)


def measure_hw_ns(features, positions, cells, neighbor_indices,
                  neighbor_distances, W, b, reps=300):
    """Time the kernel on hardware via an on-device repeat loop (amortizes
    the multi-ms axon RPC dispatch overhead). Returns per-iteration ns."""
    import time
    import jax
    from jax.sharding import Mesh, PartitionSpec, NamedSharding
    from jax.experimental.shard_map import shard_map
    from concourse import bass2jax
    from concourse.bass2jax import _bass_exec_p, partition_id_tensor

    bass2jax.install_neuronx_cc_hook()
    in_maps, R0, NCH = _host_inputs(features, positions, cells,
                                    neighbor_indices, neighbor_distances, W, b)

    def build_fn(nc, mesh, sh):
        partition_name = (nc.partition_id_tensor.name
                          if nc.partition_id_tensor else None)
        in_names, out_names, out_avals, zero_outs = [], [], [], []
        for alloc in nc.m.functions[0].allocations:
            if not isinstance(alloc, mybir.MemoryLocationSet):
                continue
            name = alloc.memorylocations[0].name
            if alloc.kind == "ExternalInput":
                if name != partition_name:
                    in_names.append(name)
            elif alloc.kind == "ExternalOutput":
                shape = tuple(alloc.tensor_shape)
                dtype = mybir.dt.np(alloc.dtype)
                out_names.append(name)
                out_avals.append(jax.core.ShapedArray(shape, dtype))
                zero_outs.append(np.zeros(shape, dtype))
        n_params = len(in_names)
        all_names = in_names + out_names
        if partition_name is not None:
            all_names = all_names + [partition_name]

        def _body(*args):
            operands = list(args)
            if partition_name is not None:
                operands.append(partition_id_tensor())
            return tuple(_bass_exec_p.bind(
                *operands, out_avals=tuple(out_avals), in_names=tuple(all_names),
                out_names=tuple(out_names), lowering_input_output_aliases=(),
                sim_require_finite=True, sim_require_nnan=True, nc=nc))

        specs_in = (PartitionSpec("core"),) * (n_params + len(out_names))
        specs_out = (PartitionSpec("core"),) * len(out_names)
        fn = jax.jit(shard_map(_body, mesh=mesh, in_specs=specs_in,
                               out_specs=specs_out, check_rep=False),
                     keep_unused=True)
        cat = [np.concatenate([np.asarray(in_maps[c][in_names[i]])
                               for c in range(NCORES)], 0)
               for i in range(n_params)]
        cat += [np.zeros((NCORES * z.shape[0], *z.shape[1:]), z.dtype)
                for z in zero_outs]
        dev = [jax.device_put(a, sh) for a in cat]
        return fn, dev

    devices = jax.devices()[:NCORES]
    mesh = Mesh(np.asarray(devices), ("core",))
    sh = NamedSharding(mesh, PartitionSpec("core"))

    def time_min(fn, dev, n=8):
        o = fn(*dev); jax.block_until_ready(o)
        best = float("inf")
        for _ in range(n):
            t0 = time.perf_counter()
            o = fn(*dev); jax.block_until_ready(o)
            best = min(best, (time.perf_counter() - t0) * 1e9)
        return best

    key1 = (R0, NCH)
    if key1 not in _CACHE:
        _CACHE[key1] = _build_nc(R0, NCH)
    fn1, dev1 = build_fn(_CACHE[key1], mesh, sh)
    t1 = time_min(fn1, dev1)
    keyr = (R0, NCH, reps)
    if keyr not in _CACHE:
        _CACHE[keyr] = _build_nc(R0, NCH, reps=reps)
    fnr, devr = build_fn(_CACHE[keyr], mesh, sh)
    tr = time_min(fnr, devr)
    return (tr - t1) / (reps - 1)


# revision 28
# speedup vs baseline: 1.2382x; 1.1164x over previous
"""Trainium2 Bass kernel for nn_LongRangeFeaturizer (Ewald sum featurizer).

Shards the 16 independent systems across 8 NeuronCores (2 systems/core).
All heavy math (charges matmul, k-space structure factors, trig, short-range
erf/cutoff coefficients, scatter, final combine) runs on-device.

Key structure (v4):
 - k-grid truncated to |n|^2 <= 15 (125 half-grid vectors + 1 background
   slot = one 128-wide k-tile); truncation error ~1.8e-3 relative, well
   under the fp16 noise floor (G ~ exp(-k^2/2)/k^2 decays brutally fast).
 - trig computed once in [K, 2N] layout (sin via table; cos = 1-2sin^2(pi v)
   on DVE); the [N, K] layout for stage 1 comes from PE transposes.
 - stage 1 computed k-major (S[k, d]); G multiply is then a per-partition
   scalar op and stage 2 consumes S directly - no extra transposes.
 - erf via tanh(a d + b d^3) (gelu identity, |err| < 4e-4): the whole kernel
   then uses one activation table set (sin/tanh/copy) = one table load.
 - Ewald self term folded into the short-range scatter matrix as diagonal
   edges with d ~ 0: sr(d->0) = -sqrt(2/pi)/sigma exactly.
 - background (k=0) term folded into the padded k slot with G = -pi s^2/V.
   The final combine is then a single multiply: out = pot * q.
 - short-range: first occurrence of each (j,i) edge goes through 512-wide
   local_scatter (8 calls, the only Pool work); duplicate occurrences
   (~530/system) are handled as 128-edge chunks: gather the edge's features
   (host-arranged columns), matmul to charges, scale rows by sr(d), then
   matmul against a host-built one-hot [slot -> i] matrix, accumulating
   straight into the potential PSUM.
"""

import sys

sys.path.insert(0, "/opt/trn_rl_repo")

import numpy as np

import concourse.bass as bass
import concourse.mybir as mybir
import concourse.tile as tile
from concourse import bacc, bass_utils

dt = mybir.dt
F32, F16, I16 = dt.float32, dt.float16, dt.int16
AF = mybir.ActivationFunctionType
AOP = mybir.AluOpType

PI = float(np.pi)
MAGIC = float(1.5 * 2**23)  # round-to-nearest-int magic constant for fp32

# Problem constants
S, N, D, E = 16, 512, 64, 16384
LCELL = 8.0
SMEAR = 1.0
EXCL = 5.0
LRWL = 1.0
PREF = 1.0
NMAX = 8
NCORES = 8
SYS_PER_CORE = S // NCORES

NSQ_CUT = 15          # keep |n|^2 <= 15; truncation err ~1.8e-3 rel
K2 = 128              # padded half-grid size (one k-tile)
DIAG_DIST = 0.01      # sr(0.01) ~= -sqrt(2/pi) = -selfc
PAD_DIST = float(EXCL)  # fcut(EXCL) = 0 -> padded slots contribute ~0
# erf(d/sqrt2) ~= tanh(TA*d + TB*d^3)  (gelu-tanh identity)
TA = 0.7978845608
TB = 0.0356774081

_CACHE = {}


def _half_kgrid():
    r = np.arange(-NMAX, NMAX + 1)
    n = np.stack(np.meshgrid(r, r, r, indexing="ij"), -1).reshape(-1, 3)
    n = n[np.any(n != 0, axis=1)]
    nsq = (n * n).sum(1)
    n = n[nsq <= NSQ_CUT]
    pos = (n[:, 0] > 0) | ((n[:, 0] == 0) & (n[:, 1] > 0)) | (
        (n[:, 0] == 0) & (n[:, 1] == 0) & (n[:, 2] > 0)
    )
    return n[pos].astype(np.int64)  # [K0, 3]


def _sr_arrange(nidx, ndist):
    """Per-system edge split: first occurrence of each (j,i) (incl. appended
    diagonal self edges) vs duplicates. Returns per-system (l0, l12) edge
    arrays and the widths R0 (l0 slots/row) and NCH (dup chunks/system)."""
    per_sys = []
    R0 = 0
    NCH = 1
    for s in range(S):
        j_t = np.concatenate([nidx[s, :, 1].astype(np.int64), np.arange(N)])
        i_t = np.concatenate([nidx[s, :, 0].astype(np.int64), np.arange(N)])
        d_t = np.concatenate([ndist[s].astype(np.float64),
                              np.full(N, DIAG_DIST)])
        cid = j_t * N + i_t
        order = np.argsort(cid, kind="stable")
        cs, js, is_, ds_ = cid[order], j_t[order], i_t[order], d_t[order]
        first = np.concatenate([[0], np.nonzero(np.diff(cs))[0] + 1])
        run_id = np.zeros(len(cs), np.int64)
        run_id[first] = 1
        run_id = np.cumsum(run_id) - 1
        occ = np.arange(len(cs)) - first[run_id]
        sel0 = occ == 0
        l0 = (js[sel0], is_[sel0], ds_[sel0])
        l12 = (js[~sel0], is_[~sel0], ds_[~sel0])
        per_sys.append((l0, l12))
        R0 = max(R0, int(np.bincount(l0[0], minlength=N).max()))
        NCH = max(NCH, -(-len(l12[0]) // 128))
    R0 += R0 % 2
    return per_sys, R0, NCH


def _build_nc(R0, NCH, reps=1):
    """Build the per-core SPMD program. NCH = dup chunks per system."""
    nc = bacc.Bacc("TRN2", target_bir_lowering=False, debug=False,
                   num_devices=NCORES)

    for val in (PI / 2,):
        t = nc.alloc_sbuf_tensor(f"constap-{val}", [128, 1], F32)
        nc.gpsimd.memset(t.ap(), val)
        nc.const_aps.aps[(F32, val)] = t.ap()
    nc.all_engine_barrier()

    def din(name, shape, d=F16):
        return nc.dram_tensor(name, shape, d, kind="ExternalInput").ap()

    SC = SYS_PER_CORE
    NCH2 = SC * NCH
    WSR = 8 * R0 + NCH2
    srd = din("srd", [128, WSR])              # f16 slot distances
    p6n6 = din("p6n6", [6, SC * N + K2])      # f16 [pT6 | nt6]
    sri = din("sri", [128, 8 * R0], I16)      # i16 l0 column indices
    nid = din("nid", [128, 256 + D])          # f16 [-I | I | WT]
    featT = din("featT", [D + 1, SC * N])     # f16 (features.T ; 1)
    gcol = din("gcol", [128, 2], F32)         # f32 G column (per-k scalar)
    featC = din("featC", [D + 1, NCH2 * 128])  # f16 dup-chunk features
    oneh = din("oneh", [128, NCH2 * N])       # f16 one-hot slot->i
    out = nc.dram_tensor("out", [SC * D, N], F32, kind="ExternalOutput").ap()

    NT = N // 128   # 4 atom tiles per system

    from contextlib import nullcontext
    with tile.TileContext(nc) as tc:
        with (
            tc.tile_pool(name="const", bufs=2) as cp,
            tc.tile_pool(name="work", bufs=2) as wp,
            tc.tile_pool(name="keep", bufs=2) as tp,
            tc.tile_pool(name="psU", bufs=2, space="PSUM") as pU,
            tc.tile_pool(name="psT", bufs=2, space="PSUM") as pT,
            tc.tile_pool(name="psH", bufs=2, space="PSUM") as pH,
            tc.For_i(0, reps, 1) if reps > 1 else nullcontext(),
        ):
            # ---- input DMAs (order = HWDGE serialization order) ----
            t_srd = cp.tile([128, WSR], F16, tag="srd")
            nc.sync.dma_start(out=t_srd[:], in_=srd[:])
            t_p6n6 = cp.tile([6, SC * N + K2], F16, tag="p6")
            nc.sync.dma_start(out=t_p6n6[:], in_=p6n6[:])
            t_sri = cp.tile([128, 8 * R0], I16, tag="sri")
            nc.sync.dma_start(out=t_sri[:], in_=sri[:])
            t_nid = cp.tile([128, 256 + D], F16, tag="nid")
            nc.sync.dma_start(out=t_nid[:], in_=nid[:])
            t_feat = cp.tile([D + 1, SC * N], F16, tag="feat")
            nc.sync.dma_start(out=t_feat[:], in_=featT[:])
            t_gcol = cp.tile([128, 2], F32, tag="gcol")
            nc.sync.dma_start(out=t_gcol[:], in_=gcol[:])
            t_fC = cp.tile([D + 1, NCH2 * 128], F16, tag="fC")
            nc.sync.dma_start(out=t_fC[:], in_=featC[:])
            t_oneh = cp.tile([128, NCH2 * N], F16, tag="oneh")
            nc.sync.dma_start(out=t_oneh[:], in_=oneh[:])

            t_pT6 = t_p6n6[:, 0:SC * N]
            t_nt6 = t_p6n6[:, SC * N:]
            t_negI = t_nid[:, 0:128]
            t_id16 = t_nid[:, 128:256]
            t_WT = t_nid[0:D + 1, 256:256 + D]

            # ---- act-table preload: Silu only lives in silu_and_others,
            #      which also holds Sin/Tanh/Copy -> one load total ----
            t_dum = wp.tile([128, 1], F16, tag="dum")
            nc.scalar.activation(t_dum[:], nc.const_aps.aps[(F32, PI / 2)],
                                 AF.Silu)

            # ---- short-range coefficients (fp16, single act table set) ----
            # sr(d) = (erf(d/sqrt2)/d) * (-0.5 - 0.5*sin(pi*d/5 + pi/2))
            # erf(d/sqrt2) ~= tanh(TA*d + TB*d^3)
            t_fc = wp.tile([128, WSR], F16, tag="srfc")
            nc.scalar.activation(t_fc[:], t_srd[:], AF.Sin,
                                 scale=float(PI / EXCL), bias=PI / 2)
            t_sq = wp.tile([128, WSR], F16, tag="srsq")
            nc.vector.tensor_tensor(out=t_sq[:], in0=t_srd[:], in1=t_srd[:],
                                    op=AOP.mult)
            t_g = wp.tile([128, WSR], F16, tag="srg")
            nc.vector.tensor_scalar(out=t_g[:], in0=t_sq[:],
                                    scalar1=TB, scalar2=TA,
                                    op0=AOP.mult, op1=AOP.add)
            t_arg = wp.tile([128, WSR], F16, tag="srarg")
            nc.vector.tensor_tensor(out=t_arg[:], in0=t_g[:], in1=t_srd[:],
                                    op=AOP.mult)
            t_erf = wp.tile([128, WSR], F16, tag="srerf")
            nc.scalar.activation(t_erf[:], t_arg[:], AF.Tanh)
            t_rec = wp.tile([128, WSR], F16, tag="srrec")
            with nc.allow_low_precision(reason="fp16 sr coefficients, 2e-2 tol"):
                nc.vector.reciprocal(t_rec[:], t_srd[:])
            t_fc2 = wp.tile([128, WSR], F16, tag="srfc2")
            nc.vector.tensor_scalar(out=t_fc2[:], in0=t_fc[:],
                                    scalar1=-0.5 * PREF, scalar2=-0.5 * PREF,
                                    op0=AOP.mult, op1=AOP.add)
            t_m1 = wp.tile([128, WSR], F16, tag="srm1")
            nc.vector.tensor_tensor(out=t_m1[:], in0=t_erf[:], in1=t_rec[:],
                                    op=AOP.mult)
            t_sr = wp.tile([128, WSR], F16, tag="srv")
            nc.vector.tensor_tensor(out=t_sr[:], in0=t_m1[:], in1=t_fc2[:],
                                    op=AOP.mult)
            # fp32 copy of the dup-chunk sr columns (activation scale AP)
            t_srf = wp.tile([128, NCH2], F32, tag="srf")
            nc.vector.tensor_copy(out=t_srf[:], in_=t_sr[:, 8 * R0:])

            # ---- 8 l0 scatters (sys-interleaved): M[blk] [128, 512] ----
            t_M = [None] * 8
            for jt in range(NT):
                for sys in range(SC):
                    blk = sys * 4 + jt
                    m = tp.tile([128, N], F16, tag=f"m_{blk}", name=f"m_{blk}")
                    csl = slice(blk * R0, (blk + 1) * R0)
                    nc.gpsimd.local_scatter(out_ap=m[:], data_ap=t_sr[:, csl],
                                            idxs_ap=t_sri[:, csl],
                                            channels=128, num_elems=N,
                                            num_idxs=R0)
                    t_M[blk] = m

            # ---- trig in KN layout: c,s [K2, 2N] f16, chunked per system ----
            # s = sin(2*pi*v), c = 1 - 2*sin(pi*v)^2 with v = u - round(u)
            t_c = tp.tile([128, SC * N], F16, tag="ckn")
            t_s = tp.tile([128, SC * N], F16, tag="skn")
            for h in range(SC):
                hsl = slice(h * N, h * N + N)
                ps_u = pU.tile([128, N], F32, tag="uc")
                nc.tensor.matmul(out=ps_u[:], lhsT=t_nt6[:],
                                 rhs=t_pT6[:, hsl], start=True, stop=False)
                t_r = wp.tile([128, N], F16, tag="rnd")
                nc.vector.tensor_scalar(out=t_r[:], in0=ps_u[:],
                                        scalar1=MAGIC, scalar2=MAGIC,
                                        op0=AOP.add, op1=AOP.subtract)
                nc.tensor.matmul(out=ps_u[:], lhsT=t_negI[:], rhs=t_r[:],
                                 start=False, stop=True)
                nc.scalar.activation(t_s[:, hsl], ps_u[:], AF.Sin,
                                     scale=2 * PI)
                ts2 = wp.tile([128, N], F16, tag="s2", bufs=2)
                nc.scalar.activation(ts2[:], ps_u[:], AF.Sin, scale=PI)
                t_sq2 = wp.tile([128, N], F16, tag="sq2")
                nc.vector.tensor_tensor(out=t_sq2[:], in0=ts2[:],
                                        in1=ts2[:], op=AOP.mult)
                nc.vector.tensor_scalar(out=t_c[:, hsl], in0=t_sq2[:],
                                        scalar1=-2.0, scalar2=1.0,
                                        op0=AOP.mult, op1=AOP.add)

            # ---- charges ----
            ps_qT = pU.tile([128, N], F32, tag="uc")
            for sys in range(SC):
                csl = slice(sys * N, sys * N + N)
                nc.tensor.matmul(out=ps_qT[sys * D:(sys + 1) * D, :],
                                 lhsT=t_WT[:], rhs=t_feat[:, csl],
                                 start=True, stop=True)
            t_qT = tp.tile([128, N], F32, tag="qtf")
            nc.scalar.activation(t_qT[:], ps_qT[:], AF.Copy)
            t_q16 = [[None] * NT for _ in range(SC)]
            for sys in range(SC):
                for nt_i in range(NT):
                    fsl = slice(sys * N + nt_i * 128, sys * N + nt_i * 128 + 128)
                    ps_q = pU.tile([128, N], F32, tag="uc")
                    nc.tensor.matmul(out=ps_q[:, 0:D], lhsT=t_feat[:, fsl],
                                     rhs=t_WT[:], start=True, stop=True)
                    tq = tp.tile([128, D], F16, tag=f"q16_{sys}_{nt_i}")
                    nc.scalar.activation(tq[:], ps_q[:, 0:D], AF.Copy)
                    t_q16[sys][nt_i] = tq
            # dup-chunk charges, scaled by sr(d) in the copy
            t_Y = []
            for gc in range(NCH2):
                ps_qc = pU.tile([128, N], F32, tag="uc")
                fsl = slice(gc * 128, (gc + 1) * 128)
                nc.tensor.matmul(out=ps_qc[:, 0:D], lhsT=t_fC[:, fsl],
                                 rhs=t_WT[:], start=True, stop=True)
                ty = tp.tile([128, D], F16, tag=f"y{gc}", name=f"y{gc}")
                nc.scalar.activation(ty[:], ps_qc[:, 0:D], AF.Copy,
                                     scale=t_srf[:, gc:gc + 1])
                t_Y.append(ty)

            # ---- per system: NK transposes -> stage1 (k-major) -> G ----
            ps_Sfull = pU.tile([128, N], F32, tag="uc")
            ps_S = ps_Sfull[:, 0:SC * 128]
            t_cs = [[None] * NT for _ in range(SC)]
            t_GS = []
            for sys in range(SC):
                scol = slice(sys * 128, sys * 128 + 128)
                # NK tiles [128(n), 256] = [c | s]
                for nt_i in range(NT):
                    nsl = slice(sys * N + nt_i * 128, sys * N + nt_i * 128 + 128)
                    ps_tr = pT.tile([128, 256], F16, tag="tr")
                    nc.tensor.transpose(out=ps_tr[:, 0:128], in_=t_c[:, nsl],
                                        identity=t_id16[:])
                    nc.tensor.transpose(out=ps_tr[:, 128:256], in_=t_s[:, nsl],
                                        identity=t_id16[:])
                    tt_ = tp.tile([128, 256], F16, tag=f"cs{sys}_{nt_i}")
                    nc.vector.tensor_copy(out=tt_[:], in_=ps_tr[:])
                    t_cs[sys][nt_i] = tt_
                # stage1 k-major: S[k, d] = sum_n c/s[n,k] q[n,d]
                # sequential accumulation groups: column-interleaved groups
                # in one psum region lose contributions (see v4 post-mortem)
                for half in range(2):
                    csl2 = slice(sys * 128 + half * D,
                                 sys * 128 + (half + 1) * D)
                    for nt_i in range(NT):
                        nc.tensor.matmul(out=ps_S[:, csl2],
                                         lhsT=t_cs[sys][nt_i][:,
                                              half * 128:(half + 1) * 128],
                                         rhs=t_q16[sys][nt_i][:],
                                         start=(nt_i == 0),
                                         stop=(nt_i == NT - 1))
                # G multiply: per-partition (= per-k) scalar
                tg = tp.tile([128, 128], F16, tag=f"gs{sys}", name=f"gs{sys}")
                nc.vector.tensor_scalar(out=tg[:], in0=ps_S[:, scol],
                                        scalar1=t_gcol[:, 0:1], scalar2=None,
                                        op0=AOP.mult)
                t_GS.append(tg)

            # ---- stage2 + dup chunks + M@q into one psum; per-sys close ----
            ps_pot = pH.tile([128, N], F32, tag="pot")
            for sys in range(SC):
                csl = slice(sys * N, sys * N + N)
                orow = slice(sys * D, (sys + 1) * D)
                nc.tensor.matmul(out=ps_pot[orow, :],
                                 lhsT=t_GS[sys][:, 0:D], rhs=t_c[:, csl],
                                 start=True, stop=False)
                nc.tensor.matmul(out=ps_pot[orow, :],
                                 lhsT=t_GS[sys][:, D:128], rhs=t_s[:, csl],
                                 start=False, stop=False)
                for ch in range(NCH):
                    gc = sys * NCH + ch
                    nc.tensor.matmul(out=ps_pot[orow, :], lhsT=t_Y[gc][:],
                                     rhs=t_oneh[:, gc * N:(gc + 1) * N],
                                     start=False, stop=False)
                for jt in range(NT):
                    nc.tensor.matmul(out=ps_pot[orow, :],
                                     lhsT=t_q16[sys][jt][:],
                                     rhs=t_M[sys * NT + jt][:],
                                     start=False, stop=(jt == NT - 1))
                # combine: out = pot * q, then output DMA per system
                t_out = wp.tile([D, N], F32, tag=f"outf{sys}",
                                name=f"outf{sys}")
                nc.vector.tensor_tensor(out=t_out[:], in0=ps_pot[orow, :],
                                        in1=t_qT[orow, :], op=AOP.mult)
                nc.sync.dma_start(out=out[orow, :], in_=t_out[:])

    nc.compile()
    return nc


def _host_inputs(features, positions, cells, neighbor_indices,
                 neighbor_distances, W, b):
    features = np.asarray(features, np.float32)
    positions = np.asarray(positions, np.float32)
    cells = np.asarray(cells, np.float32)
    nidx = np.asarray(neighbor_indices)
    ndist = np.asarray(neighbor_distances, np.float32).reshape(S, E)
    W = np.asarray(W, np.float32)
    b = np.asarray(b, np.float32)

    assert np.allclose(cells, LCELL * np.eye(3, dtype=np.float32)[None]), \
        "kernel specialized to cubic L=8 cells"

    nh = _half_kgrid()
    K0 = len(nh)
    assert K0 <= K2 - 1
    ksq = (2.0 * PI / LCELL) ** 2 * (nh * nh).sum(1).astype(np.float64)
    vol = LCELL ** 3
    bgov = PREF * float(PI * SMEAR**2 / vol)
    G = 2.0 * PREF * (4.0 * PI / ksq) * np.exp(-0.5 * SMEAR**2 * ksq) / vol
    Gpad = np.zeros(K2, np.float64)
    Gpad[:K0] = G
    Gpad[K0] = -bgov  # background term via the k=0 pad slot (c=1, s=0)
    gcol = np.zeros((128, 2), np.float32)
    gcol[:, 0] = Gpad.astype(np.float32)

    per_sys, R0, NCH = _sr_arrange(nidx, ndist)
    SC = SYS_PER_CORE
    NCH2 = SC * NCH
    WSR = 8 * R0 + NCH2

    nt3 = np.zeros((3, K2), np.float16)
    nt3[:, :K0] = nh.T.astype(np.float16)
    nt6 = np.concatenate([nt3, nt3], 0)    # [6, K2]
    WT_aug = np.concatenate([W.T, b[None, :]], 0).astype(np.float16)  # [65, 64]
    nid = np.zeros((128, 256 + D), np.float16)
    nid[:, 0:128] = -np.eye(128)
    nid[:, 128:256] = np.eye(128)
    nid[0:D + 1, 256:256 + D] = WT_aug

    in_maps = []
    for core in range(NCORES):
        s0 = core * SC
        fa = []
        p6 = []
        for s in range(s0, s0 + SC):
            f = features[s * N:(s + 1) * N].T                      # [64, 512]
            fa.append(np.concatenate([f, np.ones((1, N), np.float32)], 0))
            pf = (positions[s].T.astype(np.float64)) / LCELL       # [3, 512]
            ph = pf.astype(np.float16)
            pl = (pf - ph.astype(np.float64)).astype(np.float16)
            p6.append(np.concatenate([ph, pl], 0))                 # [6, 512]
        p6n6 = np.concatenate(p6 + [nt6], 1).astype(np.float16)

        srd_c = np.full((128, WSR), PAD_DIST, np.float16)
        sri_c = np.full((128, 8 * R0), -1, np.int16)
        featC = np.zeros((D + 1, NCH2 * 128), np.float16)
        oneh = np.zeros((128, NCH2 * N), np.float16)
        for sys_local in range(SC):
            s = s0 + sys_local
            (js, is_, ds_), (js2, is2, ds2) = per_sys[s]
            cnt = np.bincount(js, minlength=N)
            start = np.concatenate([[0], np.cumsum(cnt)[:-1]])
            slot = np.arange(len(js)) - start[js]
            blk = sys_local * 4 + js // 128
            row = js % 128
            col = blk * R0 + slot
            srd_c[row, col] = ds_.astype(np.float16)
            sri_c[row, col] = is_.astype(np.int16)
            for e in range(len(js2)):
                ch = e // 128
                sl = e % 128
                gc = sys_local * NCH + ch
                srd_c[sl, 8 * R0 + gc] = np.float16(ds2[e])
                featC[0:D, gc * 128 + sl] = \
                    features[s * N + js2[e]].astype(np.float16)
                featC[D, gc * 128 + sl] = 1.0
                oneh[sl, gc * N + is2[e]] = 1.0

        m = {
            "srd": srd_c,
            "p6n6": p6n6,
            "sri": sri_c,
            "nid": nid,
            "featT": np.concatenate(fa, 1).astype(np.float16),
            "gcol": gcol,
            "featC": featC,
            "oneh": oneh,
        }
        in_maps.append(m)
    return in_maps, R0, NCH


def kernel(features, positions, cells, neighbor_indices, neighbor_distances,
           W, b, _trace=False):
    in_maps, R0, NCH = _host_inputs(features, positions, cells,
                                    neighbor_indices, neighbor_distances, W, b)
    key = (R0, NCH)
    if key not in _CACHE:
        _CACHE[key] = _build_nc(R0, NCH)
    nc = _CACHE[key]
    res = bass_utils.run_bass_kernel_spmd(nc, in_maps,
                                          core_ids=list(range(NCORES)),
                                          trace=_trace)
    blocks = []
    for i in range(NCORES):
        o = res.results[i]["out"]  # [SC*D, N] transposed per system
        for sys in range(SYS_PER_CORE):
            blocks.append(o[sys * D:(sys + 1) * D, :].T)
    out = np.concatenate(blocks, 0)
    if _trace:
        kernel.last_result = res
    return np.ascontiguousarray(out, dtype=np.float32)


def measure_hw_ns(features, positions, cells, neighbor_indices,
                  neighbor_distances, W, b, reps=300):
    """Time the kernel on hardware via an on-device repeat loop (amortizes
    the multi-ms axon RPC dispatch overhead). Returns per-iteration ns."""
    import time
    import jax
    from jax.sharding import Mesh, PartitionSpec, NamedSharding
    from jax.experimental.shard_map import shard_map
    from concourse import bass2jax
    from concourse.bass2jax import _bass_exec_p, partition_id_tensor

    bass2jax.install_neuronx_cc_hook()
    in_maps, R0, NCH = _host_inputs(features, positions, cells,
                                    neighbor_indices, neighbor_distances, W, b)

    def build_fn(nc, mesh, sh):
        partition_name = (nc.partition_id_tensor.name
                          if nc.partition_id_tensor else None)
        in_names, out_names, out_avals, zero_outs = [], [], [], []
        for alloc in nc.m.functions[0].allocations:
            if not isinstance(alloc, mybir.MemoryLocationSet):
                continue
            name = alloc.memorylocations[0].name
            if alloc.kind == "ExternalInput":
                if name != partition_name:
                    in_names.append(name)
            elif alloc.kind == "ExternalOutput":
                shape = tuple(alloc.tensor_shape)
                dtype = mybir.dt.np(alloc.dtype)
                out_names.append(name)
                out_avals.append(jax.core.ShapedArray(shape, dtype))
                zero_outs.append(np.zeros(shape, dtype))
        n_params = len(in_names)
        all_names = in_names + out_names
        if partition_name is not None:
            all_names = all_names + [partition_name]

        def _body(*args):
            operands = list(args)
            if partition_name is not None:
                operands.append(partition_id_tensor())
            return tuple(_bass_exec_p.bind(
                *operands, out_avals=tuple(out_avals), in_names=tuple(all_names),
                out_names=tuple(out_names), lowering_input_output_aliases=(),
                sim_require_finite=True, sim_require_nnan=True, nc=nc))

        specs_in = (PartitionSpec("core"),) * (n_params + len(out_names))
        specs_out = (PartitionSpec("core"),) * len(out_names)
        fn = jax.jit(shard_map(_body, mesh=mesh, in_specs=specs_in,
                               out_specs=specs_out, check_rep=False),
                     keep_unused=True)
        cat = [np.concatenate([np.asarray(in_maps[c][in_names[i]])
                               for c in range(NCORES)], 0)
               for i in range(n_params)]
        cat += [np.zeros((NCORES * z.shape[0], *z.shape[1:]), z.dtype)
                for z in zero_outs]
        dev = [jax.device_put(a, sh) for a in cat]
        return fn, dev

    devices = jax.devices()[:NCORES]
    mesh = Mesh(np.asarray(devices), ("core",))
    sh = NamedSharding(mesh, PartitionSpec("core"))

    def time_min(fn, dev, n=8):
        o = fn(*dev); jax.block_until_ready(o)
        best = float("inf")
        for _ in range(n):
            t0 = time.perf_counter()
            o = fn(*dev); jax.block_until_ready(o)
            best = min(best, (time.perf_counter() - t0) * 1e9)
        return best

    key1 = (R0, NCH)
    if key1 not in _CACHE:
        _CACHE[key1] = _build_nc(R0, NCH)
    fn1, dev1 = build_fn(_CACHE[key1], mesh, sh)
    t1 = time_min(fn1, dev1)
    keyr = (R0, NCH, reps)
    if keyr not in _CACHE:
        _CACHE[keyr] = _build_nc(R0, NCH, reps=reps)
    fnr, devr = build_fn(_CACHE[keyr], mesh, sh)
    tr = time_min(fnr, devr)
    return (tr - t1) / (reps - 1)
